# revision 2
# baseline (speedup 1.0000x reference)
"""Trainium2 Bass kernel for nn_BinomialLoss (binomial deviance loss).

Strategy (data-parallel over 8 NeuronCores, class-sorted band layout):
  - Rows are sorted by target class on the host; per-row losses are
    permutation-invariant under the final sum, so the total is unchanged.
  - Each core's copy of the column data is ROTATED by (cR - 256) so the
    core's own 512 rows sit at columns [256, 768) — i.e. inside slabs
    0-1 — and every same-class pair of those rows lands in [0, 1024).
    The kernel is SPMD (one program, 8 cores), so both facts are
    compile-time constants.  The lhsT for the dense matmuls is sliced
    straight out of the slab tiles; no separate local-rows load.
  - Dense sim slice: each core computes sim = x_local @ x_full^T as
    [512, 4096] in fp8e4m3 with DoubleRow matmuls (2 K-planes per pass,
    216 ns / 512-col MM warm; rel-err 8.4e-4 vs 2e-2 budget).
  - Dense matmuls OPEN each PSUM group (start=True) so they only wait
    on their own slab DMAs; the rank-64 one-hot mask extension (bf16
    path, exact) accumulates -1024*[t_i==t_j] afterward over [0, 1024)
    and closes the group.  Non-band columns hold raw sim (all
    diff-class there by construction).
  - softplus(x) ~= relu(x) (error ~1e-4 on the final loss):
      neg partial: relu(w - 0.5), ONE ScalarE pass per [128, 1024] chunk
      pos partial: sum min(w, -1023.5)  (host: *-2, +const -> relu sum)
      pos count:   sum [w < -1023] == #{same & sim < 1}  (exact)
    pos/cnt run on VectorE over a per-i-tile 384-col window that
    provably contains that i-tile's same-class span (class size <= 128).
  - Per-row finalize (means, counts, total) is O(n) and runs on the host
    from a single [128, 28] fp32 accumulator DMA per core.
"""
import sys
import numpy as np

sys.path.insert(0, "/opt/trn_rl_repo")

N = 4096          # total rows
D = 512           # feature dim
NCORES = 8
R = N // NCORES   # rows per core (512)
P = 128           # partitions
NI = R // P       # i-tiles per core (4)
KS = D // P       # K planes (4)
NCLS = 64         # number of classes
SHIFT = 1024.0    # same-class mask shift
HC = 1024         # half-chunk size (2 PSUM banks; 4 bufs fill PSUM)
CHUNK = 2048      # j-chunk size (one jc = two half-chunks)
NJC = N // CHUNK  # j-chunks (2)
MMW = 512         # matmul moving width: one PSUM bank (hard limit)
LOFF = 256        # rotation puts local rows at columns [LOFF, LOFF+R)
W = 384           # pos/cnt window width
WS = (128, 256, 384, 512)  # pos/cnt window start per i-tile
NSLAB = N // MMW  # rhs DMA slabs (8)
NWARM = 5         # PE ramp warm-up matmuls (cover until xt0 lands)

_compiled = None


def _build():
    import concourse.bass as bass
    import concourse.tile as tile
    from concourse import bacc, mybir

    f32 = mybir.dt.float32
    bf16 = mybir.dt.bfloat16
    f8 = mybir.dt.float8e4
    f8e5 = mybir.dt.float8e5
    ALU = mybir.AluOpType
    ACTF = mybir.ActivationFunctionType
    DR = mybir.MatmulPerfMode.DoubleRow

    nc = bacc.Bacc("TRN2", target_bir_lowering=False, debug=False,
                   num_devices=NCORES)

    xr_ap = nc.dram_tensor("xr", [NSLAB, P, KS, MMW], f8,
                           kind="ExternalInput").ap()
    am_ap = nc.dram_tensor("am", [NCLS, R], f8e5, kind="ExternalInput").ap()
    b01_ap = nc.dram_tensor("b01", [NCLS, HC], f8e5,
                            kind="ExternalInput").ap()
    acc_ap = nc.dram_tensor("acc", [P, 28], f32,
                           kind="ExternalOutput").ap()

    with tile.TileContext(nc) as tc:
        with (
            tc.tile_pool(name="xt", bufs=1) as xt_pool,
            tc.tile_pool(name="oh", bufs=1) as oh_pool,
            tc.tile_pool(name="scr", bufs=6) as scr_pool,
            tc.tile_pool(name="misc", bufs=1) as misc_pool,
            tc.tile_pool(name="pchunk", bufs=4, space="PSUM") as pchunk_pool,
        ):
            # PE warm-up: junk matmuls (output never read) so the HAM
            # clock gate releases while the first DMAs land.
            warm_x = misc_pool.tile([P, MMW], bf16, tag="warm_x")
            nc.vector.memset(warm_x[:], 0.0)
            bias_n = misc_pool.tile([P, 1], f32, tag="bias_n")
            nc.vector.memset(bias_n[:], -0.5)
            acc = misc_pool.tile([P, 28], f32, tag="acc")
            ps_warm = pchunk_pool.tile([P, HC], f32, tag="chunk")
            for _ in range(NWARM):
                nc.tensor.matmul(ps_warm[:, 0:MMW], lhsT=warm_x[:, 0:P],
                                 rhs=warm_x[:], start=True, stop=True)

            # ---- input loads, first-needed first.  gpsimd and sync
            # ---- carry the bulk slabs round-robin; scalar only takes
            # ---- the tiny am so its ACT work can start early.
            am_t = oh_pool.tile([NCLS, R], f8e5, tag="am")
            b01_t = oh_pool.tile([NCLS, HC], f8e5, tag="b01")
            xt_t = [xt_pool.tile([P, KS, MMW], f8, tag=f"xt{s}", name=f"xt{s}")
                    for s in range(NSLAB)]
            nc.gpsimd.dma_start(out=xt_t[0][:], in_=xr_ap[0])
            nc.sync.dma_start(out=b01_t[:], in_=b01_ap[:])
            nc.scalar.dma_start(out=am_t[:], in_=am_ap[:])
            nc.gpsimd.dma_start(out=xt_t[2][:], in_=xr_ap[2])
            nc.sync.dma_start(out=xt_t[1][:], in_=xr_ap[1])
            nc.gpsimd.dma_start(out=xt_t[4][:], in_=xr_ap[4])
            nc.sync.dma_start(out=xt_t[3][:], in_=xr_ap[3])
            nc.gpsimd.dma_start(out=xt_t[6][:], in_=xr_ap[6])
            nc.sync.dma_start(out=xt_t[5][:], in_=xr_ap[5])
            nc.sync.dma_start(out=xt_t[7][:], in_=xr_ap[7])

            def dense(ps, i, slab, bank, start, stop):
                # lhsT = the core's own rows, sliced out of slabs 0-1
                col = LOFF + i * P
                lsl, loc = divmod(col, MMW)
                for s2 in range(0, KS, 2):
                    nc.tensor.matmul(
                        ps[:, bank * MMW:(bank + 1) * MMW],
                        lhsT=xt_t[lsl][:, s2:s2 + 2, loc:loc + P],
                        rhs=xt_t[slab][:, s2:s2 + 2, :],
                        start=start and s2 == 0,
                        stop=stop and s2 == KS - 2,
                        perf_mode=DR, skip_group_check=True)

            def consume_dve(ps, lo, hi, col):
                sc = scr_pool.tile([P, hi - lo], bf16, tag=f"scr{hi-lo}")
                nc.vector.tensor_scalar(
                    out=sc[:], in0=ps[:, lo:hi],
                    scalar1=0.5, scalar2=None,
                    op0=ALU.max, op1=ALU.add,
                    accum_out=acc[:, col:col + 1])

            def consume_act(ps, lo, hi, col):
                sc = scr_pool.tile([P, hi - lo], bf16, tag=f"scr{hi-lo}")
                nc.scalar.activation(
                    sc[:], ps[:, lo:hi], ACTF.Relu,
                    bias=bias_n[:], scale=1.0,
                    accum_out=acc[:, col:col + 1])

            # ---- jc0, low halves first: dense on slabs 0-1 opens the
            # ---- banks (start=True), the mask extension accumulates
            # ---- afterward over [0, 1024) and closes the group — so
            # ---- dense never waits on the am/b01 DMAs.
            for i in range(NI):
                ps = pchunk_pool.tile([P, HC], f32, tag="chunk")
                dense(ps, i, 0, 0, start=True, stop=False)
                dense(ps, i, 1, 1, start=True, stop=False)
                nc.tensor.matmul(
                    ps[:, 0:MMW], lhsT=am_t[:, i * P:(i + 1) * P],
                    rhs=b01_t[:, 0:MMW], start=False, stop=True,
                    skip_group_check=True)
                nc.tensor.matmul(
                    ps[:, MMW:HC], lhsT=am_t[:, i * P:(i + 1) * P],
                    rhs=b01_t[:, MMW:HC], start=False, stop=True,
                    skip_group_check=True)
                # The three consumers of this tile overlap in PSUM range,
                # so Tile serializes them in emission order; ScalarE is
                # the engine with the least slack, so its pass goes FIRST.
                # neg partial over the half (same-class cols give 0)
                consume_act(ps, 0, HC, 8 + i)
                # pos partial: sum min(w, -1023.5) over the i-tile window
                sc_p = scr_pool.tile([P, W], bf16, tag="scrp")
                nc.vector.tensor_scalar(
                    out=sc_p[:], in0=ps[:, WS[i]:WS[i] + W],
                    scalar1=-(SHIFT - 0.5), scalar2=None,
                    op0=ALU.min, op1=ALU.add,
                    accum_out=acc[:, 0 + i:1 + i])
                # pos count: sum [w < -1023]
                sc_c = scr_pool.tile([P, W], bf16, tag="scrp")
                nc.vector.tensor_scalar(
                    out=sc_c[:], in0=ps[:, WS[i]:WS[i] + W],
                    scalar1=-(SHIFT - 1.0), scalar2=None,
                    op0=ALU.is_lt, op1=ALU.add,
                    accum_out=acc[:, 4 + i:5 + i])

            # ---- jc0 high halves (slabs 2-3), all diff-class.
            # Engine split is BANK-ALIGNED (ScalarE + VectorE on the same
            # PSUM bank concurrently is a fatal collision) and balances
            # total engine time: ScalarE ~9.5 of 16 neg banks, VectorE
            # the rest plus the pos/cnt window passes.
            # reversed order: the VectorE-consumed tiles (i3, i2) go
            # first so ScalarE gets two chunk-times to drain its h0
            # backlog before its next PSUM tiles arrive
            for i in range(NI - 1, -1, -1):
                ps = pchunk_pool.tile([P, HC], f32, tag="chunk")
                dense(ps, i, 2, 0, start=True, stop=True)
                dense(ps, i, 3, 1, start=True, stop=True)
                if i == 0:
                    consume_act(ps, 0, HC, 12)
                elif i == 1:
                    consume_act(ps, 0, MMW, 13)
                    consume_dve(ps, MMW, HC, 14)
                else:
                    consume_dve(ps, 0, HC, 13 + i)

            # ---- jc1 (slabs 4-7), all diff-class ----
            for i in range(NI):
                for h in range(2):
                    ps = pchunk_pool.tile([P, HC], f32, tag="chunk")
                    dense(ps, i, 4 + 2 * h, 0, start=True, stop=True)
                    dense(ps, i, 5 + 2 * h, 1, start=True, stop=True)
                    if h == 0:
                        consume_act(ps, 0, HC, 17 + i)
                    elif i < NI - 1:
                        consume_dve(ps, 0, HC, 21 + i)
                    else:
                        # last tile: split so the tail is one FD512 op
                        consume_act(ps, 0, MMW, 25)
                        consume_dve(ps, MMW, HC, 21 + i)
                        # most accumulator columns are final - overlap
                        # their DMA with the last consumers
                        nc.sync.dma_start(out=acc_ap[:, 0:21], in_=acc[:, 0:21])

            nc.sync.dma_start(out=acc_ap[:, 21:28], in_=acc[:, 21:28])

    nc.compile()
    return nc


def _get_compiled():
    global _compiled
    if _compiled is None:
        _compiled = _build()
    return _compiled


def _prep(inputs):
    import ml_dtypes

    x = np.asarray(inputs["inputs"], dtype=np.float32)
    t = np.asarray(inputs["targets"]).astype(np.int64)
    assert x.shape == (N, D)

    perm = np.argsort(t, kind="stable")
    xs, ts = x[perm], t[perm]
    counts = np.bincount(ts, minlength=NCLS)
    cstart = np.concatenate([[0], np.cumsum(counts)])

    xq = xs.astype(ml_dtypes.float8_e4m3)
    # K-plane-major PE view: kv[p, s, row] = xq[row, s*128 + p]
    kv = np.ascontiguousarray(xq.T.reshape(KS, P, N).transpose(1, 0, 2))

    in_maps = []
    meta = []
    cls_ar = np.arange(NCLS)
    for c in range(NCORES):
        rows = slice(c * R, (c + 1) * R)
        tloc = ts[rows]
        rot = c * R - LOFF          # rotation offset (may be negative)
        s_c = int(cstart[tloc[0]])
        e_c = int(cstart[tloc[-1] + 1])
        assert 0 <= s_c - rot and e_c - rot <= HC, \
            f"mask band outside [0,{HC}) on core {c}"
        for i in range(NI):
            lo = int(cstart[tloc[i * P]]) - rot
            hi = int(cstart[tloc[i * P + P - 1] + 1]) - rot
            assert WS[i] <= lo and hi <= WS[i] + W, \
                f"window overflow on core {c} i-tile {i}: [{lo},{hi})"
        cols = (rot + np.arange(N)) % N   # local rows -> cols [256, 768)
        xr = kv[:, :, cols]               # [128, 4, 4096]
        xr = np.ascontiguousarray(
            xr.reshape(P, KS, NSLAB, MMW).transpose(2, 0, 1, 3))
        am = np.zeros((NCLS, R), dtype=ml_dtypes.float8_e5m2)
        am[tloc, np.arange(R)] = -SHIFT
        bcls = ts[cols[:HC]]
        b01 = (cls_ar[:, None] == bcls[None, :]).astype(ml_dtypes.float8_e5m2)
        in_maps.append({"xr": xr, "am": am, "b01": b01})
        # neg counts per local row, in acc's [partition, i-tile] layout
        ncnt = (N - counts[tloc]).astype(np.float64).reshape(NI, P).T
        meta.append(ncnt)
    return in_maps, meta


def _reduce_results(res, meta):
    total = np.float64(0.0)
    for c in range(NCORES):
        a = np.asarray(res.results[c]["acc"], dtype=np.float64)  # [128, 28]
        pos_sum = -2.0 * (a[:, 0:4] + (SHIFT - 0.5) * W)
        pos_cnt = a[:, 4:8]
        # jc0-h1 pieces per i-tile: i0=col12(ACT), i1=col13(ACT)+col14
        # (DVE FD512), i2=col15, i3=col16 (DVE FD1024); DVE max-ops
        # carry a +FD/2 offset each
        neg24 = a[:, 24] + a[:, 25] - 0.5 * MMW
        neg_relu = a[:, 8:12] + a[:, 17:21] \
            + np.stack([a[:, 21] - 0.5 * HC, a[:, 22] - 0.5 * HC,
                        a[:, 23] - 0.5 * HC, neg24], axis=1) \
            + np.stack([
                a[:, 12],
                a[:, 13] + a[:, 14] - 0.5 * MMW,
                a[:, 15] - 0.5 * HC,
                a[:, 16] - 0.5 * HC,
            ], axis=1)
        pos_mean = pos_sum / np.maximum(pos_cnt, 1.0)
        neg_mean = 25.0 * neg_relu / meta[c]
        total += float(np.sum(pos_mean + neg_mean))
    return np.float32(total / N)


def kernel(**inputs) -> np.ndarray:
    from concourse.bass_utils import run_bass_kernel_spmd

    nc = _get_compiled()
    in_maps, meta = _prep(inputs)
    res = run_bass_kernel_spmd(nc, in_maps, list(range(NCORES)))
    return _reduce_results(res, meta)


def kernel_timed(**inputs):
    """Like kernel(), but NTFF-profiles core 0 and returns
    (loss, exec_time_ns, profile_json_path)."""
    from concourse.bass_utils import run_bass_kernel_spmd

    nc = _get_compiled()
    in_maps, meta = _prep(inputs)
    run_bass_kernel_spmd(nc, in_maps, list(range(NCORES)))  # warm NEFF cache
    res = run_bass_kernel_spmd(nc, in_maps, list(range(NCORES)), trace=True)
    return _reduce_results(res, meta), res.exec_time_ns, res.profile_json


# revision 9
# speedup vs baseline: 1.0967x; 1.0967x over previous
"""Trainium2 Bass kernel for nn_BinomialLoss (binomial deviance loss).

Strategy (data-parallel over 8 NeuronCores, class-sorted band layout):
  - Rows are sorted by target class on the host; per-row losses are
    permutation-invariant under the final sum, so the total is unchanged.
  - Each core's copy of the column data is ROTATED by (cR - 256) so
    every same-class pair of the core's 512 rows lands in columns
    [0, 1024).  The kernel is SPMD (one program, 8 cores), so the band
    position must be a compile-time constant.
  - Dense sim slice: each core computes sim = x_local @ x_full^T as
    [512, 4096] in fp8e4m3 with DoubleRow matmuls (2 K-planes per pass,
    216 ns / 512-col MM warm; rel-err 8.4e-4 vs 2e-2 budget).  lhsT
    lives in four dedicated 64 KB tiles (separate SBUF region from the
    rhs slabs - same-region operands serialize LDWEIGHTS against the
    moving stream and cost ~20-140% per MM).
  - The chip's clock/DMA ramp makes the FIRST ~256KB on any queue land
    only ~13 us in; the gating inputs are therefore split small (xl as
    4x64KB, slab0 as 2x128KB) so dense work can start ~11 us, with junk
    warm-up matmuls covering the ramp before that.  Dense matmuls OPEN
    each PSUM group (start=True); the rank-64 one-hot mask extension
    accumulates -1024*[t_i==t_j] over [0, 1024) afterward and closes
    the group, so dense never waits on the am/b01 DMAs.
  - softplus(x) ~= relu(x) (error ~1e-4 on the final loss):
      neg partial: relu(w - 0.5), ONE ScalarE pass per [128, 1024] chunk
      pos partial: sum min(w, -1023.5)  (host: *-2, +const -> relu sum)
      pos count:   sum [w < -1023] == #{same & sim < 1}  (exact)
    pos/cnt run on VectorE over a per-i-tile 384-col window that
    provably contains that i-tile's same-class span (class size <= 128).
  - Per-row finalize (means, counts, total) is O(n) and runs on the host
    from a single [128, 28] fp32 accumulator DMA per core.
"""
import sys
import numpy as np

sys.path.insert(0, "/opt/trn_rl_repo")

N = 4096          # total rows
D = 512           # feature dim
NCORES = 8
R = N // NCORES   # rows per core (512)
P = 128           # partitions
NI = R // P       # i-tiles per core (4)
KS = D // P       # K planes (4)
NCLS = 64         # number of classes
SHIFT = 1024.0    # same-class mask shift
HC = 1024         # half-chunk size (2 PSUM banks; 4 bufs fill PSUM)
CHUNK = 2048      # j-chunk size (one jc = two half-chunks)
NJC = N // CHUNK  # j-chunks (2)
MMW = 512         # matmul moving width: one PSUM bank (hard limit)
HW_ = 256         # half-slab width for the split slab 0
LOFF = 256        # rotation: band at [0, 1024), windows per WS below
W = 384           # pos/cnt window width
WS = (128, 256, 384, 512)  # pos/cnt window start per i-tile
NSLAB = N // MMW  # rhs DMA slabs (8)
NWARM = 6         # PE ramp warm-up matmuls (cover until xt0a lands)

_compiled = None


def _build():
    import concourse.bass as bass
    import concourse.tile as tile
    from concourse import bacc, mybir

    f32 = mybir.dt.float32
    bf16 = mybir.dt.bfloat16
    f8 = mybir.dt.float8e4
    f8e5 = mybir.dt.float8e5
    ALU = mybir.AluOpType
    ACTF = mybir.ActivationFunctionType
    DR = mybir.MatmulPerfMode.DoubleRow

    nc = bacc.Bacc("TRN2", target_bir_lowering=False, debug=False,
                   num_devices=NCORES)

    xr0a_ap = nc.dram_tensor("xr0a", [P, 2, MMW], f8,
                             kind="ExternalInput").ap()
    xr0b_ap = nc.dram_tensor("xr0b", [P, 2, MMW], f8,
                             kind="ExternalInput").ap()
    xrr_ap = nc.dram_tensor("xrr", [NSLAB - 1, P, KS, MMW], f8,
                            kind="ExternalInput").ap()
    xl_ap = nc.dram_tensor("xl", [NI, P, KS, P], f8,
                           kind="ExternalInput").ap()
    am_ap = nc.dram_tensor("am", [NCLS, R], f8e5, kind="ExternalInput").ap()
    b01_ap = nc.dram_tensor("b01", [NCLS, HC], f8e5,
                            kind="ExternalInput").ap()
    acc_ap = nc.dram_tensor("acc", [P, 28], f32,
                           kind="ExternalOutput").ap()

    with tile.TileContext(nc) as tc:
        with (
            tc.tile_pool(name="xt", bufs=1) as xt_pool,
            tc.tile_pool(name="xl", bufs=1) as xl_pool,
            tc.tile_pool(name="oh", bufs=1) as oh_pool,
            tc.tile_pool(name="scr", bufs=6) as scr_pool,
            tc.tile_pool(name="misc", bufs=1) as misc_pool,
            tc.tile_pool(name="pchunk", bufs=4, space="PSUM") as pchunk_pool,
        ):
            # PE warm-up: junk matmuls (output never read) so the HAM
            # clock gate releases while the first DMAs land.
            warm_x = misc_pool.tile([P, MMW], bf16, tag="warm_x")
            nc.vector.memset(warm_x[:], 0.0)
            bias_n = misc_pool.tile([P, 1], f32, tag="bias_n")
            nc.vector.memset(bias_n[:], -0.5)
            acc = misc_pool.tile([P, 28], f32, tag="acc")
            ps_warm = pchunk_pool.tile([P, HC], f32, tag="chunk")
            for _ in range(NWARM):
                nc.tensor.matmul(ps_warm[:, 0:MMW], lhsT=warm_x[:, 0:P],
                                 rhs=warm_x[:], start=True, stop=True)

            # ---- input loads, first-needed first; the clock ramp makes
            # ---- early transfers slow, so the gating pieces are small.
            xl_t = [xl_pool.tile([P, KS, P], f8, tag=f"xl{i}", name=f"xl{i}")
                    for i in range(NI)]
            am_t = oh_pool.tile([NCLS, R], f8e5, tag="am")
            b01_t = oh_pool.tile([NCLS, HC], f8e5, tag="b01")
            xt0a_t = xt_pool.tile([P, 2, MMW], f8, tag="xt0a", name="xt0a")
            xt0b_t = xt_pool.tile([P, 2, MMW], f8, tag="xt0b", name="xt0b")
            xt_t = [None] + [
                xt_pool.tile([P, KS, MMW], f8, tag=f"xt{s}", name=f"xt{s}")
                for s in range(1, NSLAB)]
            # gpsimd (fastest queue): the tiny gating pieces, in need
            # order; sync: the slab-0 halves + early slabs; scalar
            # (slowest queue, measured ~4x less early bandwidth): only
            # late bulk.
            nc.gpsimd.dma_start(out=xl_t[0][:], in_=xl_ap[0])
            nc.sync.dma_start(out=xt0a_t[:], in_=xr0a_ap[:])
            nc.gpsimd.dma_start(out=xl_t[1][:], in_=xl_ap[1])
            nc.gpsimd.dma_start(out=xl_t[2][:], in_=xl_ap[2])
            nc.sync.dma_start(out=xt0b_t[:], in_=xr0b_ap[:])
            nc.gpsimd.dma_start(out=xl_t[3][:], in_=xl_ap[3])
            nc.gpsimd.dma_start(out=am_t[:], in_=am_ap[:])
            nc.sync.dma_start(out=xt_t[1][:], in_=xrr_ap[0])
            nc.gpsimd.dma_start(out=b01_t[:], in_=b01_ap[:])
            nc.gpsimd.dma_start(out=xt_t[2][:], in_=xrr_ap[1])
            nc.sync.dma_start(out=xt_t[3][:], in_=xrr_ap[2])
            nc.scalar.dma_start(out=xt_t[4][:], in_=xrr_ap[3])
            nc.sync.dma_start(out=xt_t[5][:], in_=xrr_ap[4])
            nc.scalar.dma_start(out=xt_t[6][:], in_=xrr_ap[5])
            nc.sync.dma_start(out=xt_t[7][:], in_=xrr_ap[6])

            def dense(ps, i, slab, bank, start, stop):
                for s2 in range(0, KS, 2):
                    nc.tensor.matmul(
                        ps[:, bank * MMW:(bank + 1) * MMW],
                        lhsT=xl_t[i][:, s2:s2 + 2, :],
                        rhs=xt_t[slab][:, s2:s2 + 2, :],
                        start=start and s2 == 0,
                        stop=stop and s2 == KS - 2,
                        perf_mode=DR, skip_group_check=True)

            def dense_h(ps, i, xth, s2, start):
                # one K-plane-pair pass of slab 0 (split for early DMA)
                nc.tensor.matmul(
                    ps[:, 0:MMW],
                    lhsT=xl_t[i][:, s2:s2 + 2, :],
                    rhs=xth[:, :, :],
                    start=start, stop=False,
                    perf_mode=DR, skip_group_check=True)

            def consume_dve(ps, lo, hi, col):
                sc = scr_pool.tile([P, hi - lo], bf16, tag=f"scr{hi-lo}")
                nc.vector.tensor_scalar(
                    out=sc[:], in0=ps[:, lo:hi],
                    scalar1=0.5, scalar2=None,
                    op0=ALU.max, op1=ALU.add,
                    accum_out=acc[:, col:col + 1])

            def consume_act(ps, lo, hi, col):
                sc = scr_pool.tile([P, hi - lo], bf16, tag=f"scr{hi-lo}")
                nc.scalar.activation(
                    sc[:], ps[:, lo:hi], ACTF.Relu,
                    bias=bias_n[:], scale=1.0,
                    accum_out=acc[:, col:col + 1])

            # ---- jc0, low halves first.  Emission follows input
            # ---- arrival order: all i-tiles' K01 pass of slab 0, then
            # ---- the K23 pass, then slab 1 + mask + consumers per
            # ---- i-tile.  Dense opens each PSUM range (start=True);
            # ---- the mask closes [0, 896) afterward, so dense never
            # ---- waits on am/b01.  (Band provably ends < 896.)
            ps_l = []
            for i in range(NI):
                ps = pchunk_pool.tile([P, HC], f32, tag="chunk")
                ps_l.append(ps)
                dense_h(ps, i, xt0a_t, 0, start=True)
            for i in range(NI):
                dense_h(ps_l[i], i, xt0b_t, 2, start=False)
            for i in range(NI):
                ps = ps_l[i]
                dense(ps, i, 1, 1, start=True, stop=False)
                nc.tensor.matmul(
                    ps[:, 0:MMW], lhsT=am_t[:, i * P:(i + 1) * P],
                    rhs=b01_t[:, 0:MMW], start=False, stop=True,
                    skip_group_check=True)
                nc.tensor.matmul(
                    ps[:, MMW:896], lhsT=am_t[:, i * P:(i + 1) * P],
                    rhs=b01_t[:, MMW:896], start=False, stop=True,
                    skip_group_check=True)
                # The three consumers of this tile overlap in PSUM range,
                # so Tile serializes them in emission order; ScalarE is
                # the engine with the least slack, so its pass goes FIRST.
                # neg partial over the half (same-class cols give 0)
                consume_act(ps, 0, HC, 8 + i)
                # pos partial: sum min(w, -1023.5) over the i-tile window
                sc_p = scr_pool.tile([P, W], bf16, tag="scrp")
                nc.vector.tensor_scalar(
                    out=sc_p[:], in0=ps[:, WS[i]:WS[i] + W],
                    scalar1=-(SHIFT - 0.5), scalar2=None,
                    op0=ALU.min, op1=ALU.add,
                    accum_out=acc[:, 0 + i:1 + i])
                # pos count: sum [w < -1023]
                sc_c = scr_pool.tile([P, W], bf16, tag="scrp")
                nc.vector.tensor_scalar(
                    out=sc_c[:], in0=ps[:, WS[i]:WS[i] + W],
                    scalar1=-(SHIFT - 1.0), scalar2=None,
                    op0=ALU.is_lt, op1=ALU.add,
                    accum_out=acc[:, 4 + i:5 + i])

            # ---- jc0 high halves (slabs 2-3), all diff-class.
            # Engine split is BANK-ALIGNED (ScalarE + VectorE on the same
            # PSUM bank concurrently is a fatal collision) and balances
            # total engine time: ScalarE ~9.5 of 16 neg banks, VectorE
            # the rest plus the pos/cnt window passes.
            # reversed order: the VectorE-consumed tiles (i3, i2) go
            # first so ScalarE gets two chunk-times to drain its h0
            # backlog before its next PSUM tiles arrive
            for i in range(NI - 1, -1, -1):
                ps = pchunk_pool.tile([P, HC], f32, tag="chunk")
                dense(ps, i, 2, 0, start=True, stop=True)
                dense(ps, i, 3, 1, start=True, stop=True)
                if i == 0:
                    consume_act(ps, 0, HC, 12)
                elif i == 1:
                    consume_act(ps, 0, MMW, 13)
                    consume_dve(ps, MMW, HC, 14)
                else:
                    consume_dve(ps, 0, HC, 13 + i)

            # ---- jc1 (slabs 4-7), all diff-class ----
            for i in range(NI):
                for h in range(2):
                    ps = pchunk_pool.tile([P, HC], f32, tag="chunk")
                    dense(ps, i, 4 + 2 * h, 0, start=True, stop=True)
                    dense(ps, i, 5 + 2 * h, 1, start=True, stop=True)
                    if h == 0:
                        consume_act(ps, 0, HC, 17 + i)
                    elif i < NI - 1:
                        consume_dve(ps, 0, HC, 21 + i)
                    else:
                        # last tile: split so the tail is one FD512 op
                        consume_act(ps, 0, MMW, 25)
                        consume_dve(ps, MMW, HC, 21 + i)
                        # most accumulator columns are final - overlap
                        # their DMA with the last consumers
                        nc.sync.dma_start(out=acc_ap[:, 0:21], in_=acc[:, 0:21])

            nc.sync.dma_start(out=acc_ap[:, 21:28], in_=acc[:, 21:28])

    nc.compile()
    return nc


def _get_compiled():
    global _compiled
    if _compiled is None:
        _compiled = _build()
    return _compiled


def _prep(inputs):
    import ml_dtypes

    x = np.asarray(inputs["inputs"], dtype=np.float32)
    t = np.asarray(inputs["targets"]).astype(np.int64)
    assert x.shape == (N, D)

    perm = np.argsort(t, kind="stable")
    xs, ts = x[perm], t[perm]
    counts = np.bincount(ts, minlength=NCLS)
    cstart = np.concatenate([[0], np.cumsum(counts)])

    xq = xs.astype(ml_dtypes.float8_e4m3)
    # K-plane-major PE view: kv[p, s, row] = xq[row, s*128 + p]
    kv = np.ascontiguousarray(xq.T.reshape(KS, P, N).transpose(1, 0, 2))

    in_maps = []
    meta = []
    cls_ar = np.arange(NCLS)
    for c in range(NCORES):
        rows = slice(c * R, (c + 1) * R)
        tloc = ts[rows]
        rot = c * R - LOFF          # rotation offset (may be negative)
        s_c = int(cstart[tloc[0]])
        e_c = int(cstart[tloc[-1] + 1])
        assert 0 <= s_c - rot and e_c - rot <= 896, \
            f"mask band outside [0,896) on core {c}"
        for i in range(NI):
            lo = int(cstart[tloc[i * P]]) - rot
            hi = int(cstart[tloc[i * P + P - 1] + 1]) - rot
            assert WS[i] <= lo and hi <= WS[i] + W, \
                f"window overflow on core {c} i-tile {i}: [{lo},{hi})"
        cols = (rot + np.arange(N)) % N
        xrr_full = kv[:, :, cols]         # [128, 4, 4096]
        xr0a = np.ascontiguousarray(xrr_full[:, 0:2, 0:MMW])
        xr0b = np.ascontiguousarray(xrr_full[:, 2:4, 0:MMW])
        xrr = np.ascontiguousarray(
            xrr_full.reshape(P, KS, NSLAB, MMW).transpose(2, 0, 1, 3)[1:])
        xl = np.ascontiguousarray(
            kv[:, :, c * R:(c + 1) * R].reshape(P, KS, NI, P)
            .transpose(2, 0, 1, 3))
        am = np.zeros((NCLS, R), dtype=ml_dtypes.float8_e5m2)
        am[tloc, np.arange(R)] = -SHIFT
        bcls = ts[cols[:HC]]
        b01 = (cls_ar[:, None] == bcls[None, :]).astype(ml_dtypes.float8_e5m2)
        in_maps.append({"xr0a": xr0a, "xr0b": xr0b, "xrr": xrr, "xl": xl,
                        "am": am, "b01": b01})
        # neg counts per local row, in acc's [partition, i-tile] layout
        ncnt = (N - counts[tloc]).astype(np.float64).reshape(NI, P).T
        meta.append(ncnt)
    return in_maps, meta


def _reduce_results(res, meta):
    total = np.float64(0.0)
    for c in range(NCORES):
        a = np.asarray(res.results[c]["acc"], dtype=np.float64)  # [128, 28]
        pos_sum = -2.0 * (a[:, 0:4] + (SHIFT - 0.5) * W)
        pos_cnt = a[:, 4:8]
        # jc0-h1 pieces per i-tile: i0=col12(ACT), i1=col13(ACT)+col14
        # (DVE FD512), i2=col15, i3=col16 (DVE FD1024); DVE max-ops
        # carry a +FD/2 offset each
        neg24 = a[:, 24] + a[:, 25] - 0.5 * MMW
        neg_relu = a[:, 8:12] + a[:, 17:21] \
            + np.stack([a[:, 21] - 0.5 * HC, a[:, 22] - 0.5 * HC,
                        a[:, 23] - 0.5 * HC, neg24], axis=1) \
            + np.stack([
                a[:, 12],
                a[:, 13] + a[:, 14] - 0.5 * MMW,
                a[:, 15] - 0.5 * HC,
                a[:, 16] - 0.5 * HC,
            ], axis=1)
        pos_mean = pos_sum / np.maximum(pos_cnt, 1.0)
        neg_mean = 25.0 * neg_relu / meta[c]
        total += float(np.sum(pos_mean + neg_mean))
    return np.float32(total / N)


def kernel(**inputs) -> np.ndarray:
    from concourse.bass_utils import run_bass_kernel_spmd

    nc = _get_compiled()
    in_maps, meta = _prep(inputs)
    res = run_bass_kernel_spmd(nc, in_maps, list(range(NCORES)))
    return _reduce_results(res, meta)


def kernel_timed(**inputs):
    """Like kernel(), but NTFF-profiles core 0 and returns
    (loss, exec_time_ns, profile_json_path)."""
    from concourse.bass_utils import run_bass_kernel_spmd

    nc = _get_compiled()
    in_maps, meta = _prep(inputs)
    run_bass_kernel_spmd(nc, in_maps, list(range(NCORES)))  # warm NEFF cache
    res = run_bass_kernel_spmd(nc, in_maps, list(range(NCORES)), trace=True)
    return _reduce_results(res, meta), res.exec_time_ns, res.profile_json


# revision 10
# speedup vs baseline: 1.2104x; 1.1037x over previous
"""Trainium2 Bass kernel for nn_BinomialLoss (binomial deviance loss).

Strategy (data-parallel over 8 NeuronCores, class-sorted band layout):
  - Rows are sorted by target class on the host; per-row losses are
    permutation-invariant under the final sum, so the total is unchanged.
  - Each core's copy of the column data is ROTATED by (cR - 256) so
    every same-class pair of the core's 512 rows lands in columns
    [0, 1024).  The kernel is SPMD (one program, 8 cores), so the band
    position must be a compile-time constant.
  - Dense sim slice: each core computes sim = x_local @ x_full^T as
    [512, 4096] in fp8e4m3 with DoubleRow matmuls (2 K-planes per pass,
    216 ns / 512-col MM warm; rel-err 8.4e-4 vs 2e-2 budget).  lhsT
    lives in four dedicated 64 KB tiles (separate SBUF region from the
    rhs slabs - same-region operands serialize LDWEIGHTS against the
    moving stream).
  - The whole core is clock-gated ~2x for the first ~10 us of activity
    (HAM), and the DMA queues crawl (~20-25 GB/s each) until ~15 us.
    Schedule shape: a few junk warm-ups, then the MASK matmuls (the
    rank-64 one-hot extension adding -1024*[t_i==t_j] over [0, 1024),
    fed by only 96 KB of input split first across all three queues)
    fill the cold window with real work; dense accumulates on top as
    its slabs arrive (small first pieces: 4x64KB lhsT, slab 0 split in
    two K-pair halves), with the late bulk as big column-contiguous
    transfers.  Matmul PSUM ranges: mask opens (start=True), dense
    accumulates, last dense pass closes.
  - softplus(x) ~= relu(x) (error ~1e-4 on the final loss):
      neg partial: relu(w - 0.5), ONE ScalarE pass per [128, 1024] chunk
      pos partial: sum min(w, -1023.5)  (host: *-2, +const -> relu sum)
      pos count:   sum [w < -1023] == #{same & sim < 1}  (exact)
    pos/cnt run on VectorE over a per-i-tile 384-col window that
    provably contains that i-tile's same-class span (class size <= 128).
  - Per-row finalize (means, counts, total) is O(n) and runs on the host
    from a single [128, 28] fp32 accumulator DMA per core.
"""
import sys
import numpy as np

sys.path.insert(0, "/opt/trn_rl_repo")

N = 4096          # total rows
D = 512           # feature dim
NCORES = 8
R = N // NCORES   # rows per core (512)
P = 128           # partitions
NI = R // P       # i-tiles per core (4)
KS = D // P       # K planes (4)
NCLS = 64         # number of classes
SHIFT = 1024.0    # same-class mask shift
HC = 1024         # half-chunk size (2 PSUM banks; 4 bufs fill PSUM)
CHUNK = 2048      # j-chunk size (one jc = two half-chunks)
NJC = N // CHUNK  # j-chunks (2)
MMW = 512         # matmul moving width: one PSUM bank (hard limit)
LOFF = 256        # rotation: band at [0, 1024), windows per WS below
W = 384           # pos/cnt window width
WS = (128, 256, 384, 512)  # pos/cnt window start per i-tile
NSLAB = N // MMW  # rhs DMA slabs (8)
NWARM = 5         # PE ramp warm-up matmuls (cover until am/b01 land)

_compiled = None


def _build():
    import concourse.bass as bass
    import concourse.tile as tile
    from concourse import bacc, mybir

    f32 = mybir.dt.float32
    bf16 = mybir.dt.bfloat16
    f8 = mybir.dt.float8e4
    f8e5 = mybir.dt.float8e5
    ALU = mybir.AluOpType
    ACTF = mybir.ActivationFunctionType
    DR = mybir.MatmulPerfMode.DoubleRow

    nc = bacc.Bacc("TRN2", target_bir_lowering=False, debug=False,
                   num_devices=NCORES)

    xr0a_ap = nc.dram_tensor("xr0a", [P, 2, MMW], f8,
                             kind="ExternalInput").ap()
    xr0b_ap = nc.dram_tensor("xr0b", [P, 2, MMW], f8,
                             kind="ExternalInput").ap()
    xr1_ap = nc.dram_tensor("xr1", [P, KS, MMW], f8,
                            kind="ExternalInput").ap()
    xr23_ap = nc.dram_tensor("xr23", [P, KS, 2 * MMW], f8,
                             kind="ExternalInput").ap()
    xr45_ap = nc.dram_tensor("xr45", [P, KS, 2 * MMW], f8,
                             kind="ExternalInput").ap()
    xr67_ap = nc.dram_tensor("xr67", [P, KS, 2 * MMW], f8,
                             kind="ExternalInput").ap()
    xl_ap = nc.dram_tensor("xl", [NI, P, KS, P], f8,
                           kind="ExternalInput").ap()
    am_ap = nc.dram_tensor("am", [NCLS, R], f8e5, kind="ExternalInput").ap()
    b01a_ap = nc.dram_tensor("b01a", [NCLS, MMW], f8e5,
                             kind="ExternalInput").ap()
    b01b_ap = nc.dram_tensor("b01b", [NCLS, MMW], f8e5,
                             kind="ExternalInput").ap()
    acc_ap = nc.dram_tensor("acc", [P, 28], f32,
                           kind="ExternalOutput").ap()

    with tile.TileContext(nc) as tc:
        with (
            tc.tile_pool(name="xt", bufs=1) as xt_pool,
            tc.tile_pool(name="xl", bufs=1) as xl_pool,
            tc.tile_pool(name="oh", bufs=1) as oh_pool,
            tc.tile_pool(name="scr", bufs=6) as scr_pool,
            tc.tile_pool(name="misc", bufs=1) as misc_pool,
            tc.tile_pool(name="pchunk", bufs=4, space="PSUM") as pchunk_pool,
        ):
            # PE warm-up: junk matmuls (output never read) so the HAM
            # clock gate releases while the first DMAs land.
            warm_x = misc_pool.tile([P, MMW], bf16, tag="warm_x")
            nc.vector.memset(warm_x[:], 0.0)
            bias_n = misc_pool.tile([P, 1], f32, tag="bias_n")
            nc.vector.memset(bias_n[:], -0.5)
            acc = misc_pool.tile([P, 28], f32, tag="acc")
            ps_warm = pchunk_pool.tile([P, HC], f32, tag="chunk")
            for _ in range(NWARM):
                nc.tensor.matmul(ps_warm[:, 0:MMW], lhsT=warm_x[:, 0:P],
                                 rhs=warm_x[:], start=True, stop=True)

            # ---- input loads, ordered by need across the three
            # ---- queues; mask inputs (96 KB total) go first-position
            # ---- everywhere so the mask matmuls can fill the clock-
            # ---- ramp window with real work.
            xl_t = [xl_pool.tile([P, KS, P], f8, tag=f"xl{i}", name=f"xl{i}")
                    for i in range(NI)]
            am_t = oh_pool.tile([NCLS, R], f8e5, tag="am")
            b01a_t = oh_pool.tile([NCLS, MMW], f8e5, tag="b01a")
            b01b_t = oh_pool.tile([NCLS, MMW], f8e5, tag="b01b")
            xt0a_t = xt_pool.tile([P, 2, MMW], f8, tag="xt0a", name="xt0a")
            xt0b_t = xt_pool.tile([P, 2, MMW], f8, tag="xt0b", name="xt0b")
            xt1_t = xt_pool.tile([P, KS, MMW], f8, tag="xt1", name="xt1")
            xt23_t = xt_pool.tile([P, KS, 2 * MMW], f8, tag="xt23",
                                  name="xt23")
            xt45_t = xt_pool.tile([P, KS, 2 * MMW], f8, tag="xt45",
                                  name="xt45")
            xt67_t = xt_pool.tile([P, KS, 2 * MMW], f8, tag="xt67",
                                  name="xt67")
            nc.gpsimd.dma_start(out=am_t[:], in_=am_ap[:])
            nc.sync.dma_start(out=b01a_t[:], in_=b01a_ap[:])
            nc.scalar.dma_start(out=b01b_t[:], in_=b01b_ap[:])
            nc.gpsimd.dma_start(out=xl_t[0][:], in_=xl_ap[0])
            nc.sync.dma_start(out=xt0a_t[:], in_=xr0a_ap[:])
            nc.gpsimd.dma_start(out=xl_t[1][:], in_=xl_ap[1])
            nc.scalar.dma_start(out=xl_t[2][:], in_=xl_ap[2])
            nc.sync.dma_start(out=xt0b_t[:], in_=xr0b_ap[:])
            nc.scalar.dma_start(out=xl_t[3][:], in_=xl_ap[3])
            nc.gpsimd.dma_start(out=xt1_t[:], in_=xr1_ap[:])
            nc.sync.dma_start(out=xt23_t[:], in_=xr23_ap[:])
            nc.scalar.dma_start(out=xt45_t[:], in_=xr45_ap[:])
            nc.gpsimd.dma_start(out=xt67_t[:], in_=xr67_ap[:])

            def dense(ps, i, xt, off, bank, start, stop):
                # xt: a slab tile [P, KS, w]; off: column offset in it
                for s2 in range(0, KS, 2):
                    nc.tensor.matmul(
                        ps[:, bank * MMW:(bank + 1) * MMW],
                        lhsT=xl_t[i][:, s2:s2 + 2, :],
                        rhs=xt[:, s2:s2 + 2, off:off + MMW],
                        start=start and s2 == 0,
                        stop=stop and s2 == KS - 2,
                        perf_mode=DR, skip_group_check=True)

            def consume_dve(ps, lo, hi, col):
                sc = scr_pool.tile([P, hi - lo], bf16, tag=f"scr{hi-lo}")
                nc.vector.tensor_scalar(
                    out=sc[:], in0=ps[:, lo:hi],
                    scalar1=0.5, scalar2=None,
                    op0=ALU.max, op1=ALU.add,
                    accum_out=acc[:, col:col + 1])

            def consume_act(ps, lo, hi, col):
                sc = scr_pool.tile([P, hi - lo], bf16, tag=f"scr{hi-lo}")
                nc.scalar.activation(
                    sc[:], ps[:, lo:hi], ACTF.Relu,
                    bias=bias_n[:], scale=1.0,
                    accum_out=acc[:, col:col + 1])

            # ---- jc0, low halves.  Mask matmuls first (they open both
            # ---- banks, start=True, and run during the clock ramp);
            # ---- dense accumulates in input-arrival order: the slab-0
            # ---- K01 pass for all i-tiles, then K23, then slab 1 +
            # ---- consumers per i-tile.
            ps_l = []
            for i in range(NI):
                ps = pchunk_pool.tile([P, HC], f32, tag="chunk")
                ps_l.append(ps)
                nc.tensor.matmul(
                    ps[:, 0:MMW], lhsT=am_t[:, i * P:(i + 1) * P],
                    rhs=b01a_t[:], start=True, stop=False,
                    skip_group_check=True)
            for i in range(NI):
                nc.tensor.matmul(
                    ps_l[i][:, MMW:HC], lhsT=am_t[:, i * P:(i + 1) * P],
                    rhs=b01b_t[:], start=True, stop=False,
                    skip_group_check=True)
            for i in range(NI):
                nc.tensor.matmul(
                    ps_l[i][:, 0:MMW],
                    lhsT=xl_t[i][:, 0:2, :], rhs=xt0a_t[:, :, :],
                    start=False, stop=False,
                    perf_mode=DR, skip_group_check=True)
            for i in range(NI):
                nc.tensor.matmul(
                    ps_l[i][:, 0:MMW],
                    lhsT=xl_t[i][:, 2:4, :], rhs=xt0b_t[:, :, :],
                    start=False, stop=True,
                    perf_mode=DR, skip_group_check=True)
            for i in range(NI):
                ps = ps_l[i]
                dense(ps, i, xt1_t, 0, 1, start=False, stop=True)
                # The three consumers of this tile overlap in PSUM range,
                # so Tile serializes them in emission order; ScalarE is
                # the engine with the least slack, so its pass goes FIRST.
                # neg partial over the half (same-class cols give 0)
                consume_act(ps, 0, HC, 8 + i)
                # pos partial: sum min(w, -1023.5) over the i-tile window
                sc_p = scr_pool.tile([P, W], bf16, tag="scrp")
                nc.vector.tensor_scalar(
                    out=sc_p[:], in0=ps[:, WS[i]:WS[i] + W],
                    scalar1=-(SHIFT - 0.5), scalar2=None,
                    op0=ALU.min, op1=ALU.add,
                    accum_out=acc[:, 0 + i:1 + i])
                # pos count: sum [w < -1023]
                sc_c = scr_pool.tile([P, W], bf16, tag="scrp")
                nc.vector.tensor_scalar(
                    out=sc_c[:], in0=ps[:, WS[i]:WS[i] + W],
                    scalar1=-(SHIFT - 1.0), scalar2=None,
                    op0=ALU.is_lt, op1=ALU.add,
                    accum_out=acc[:, 4 + i:5 + i])

            # ---- jc0 high halves (slabs 2-3), all diff-class.
            # Engine split is BANK-ALIGNED (ScalarE + VectorE on the same
            # PSUM bank concurrently is a fatal collision) and balances
            # total engine time: ScalarE ~9.5 of 16 neg banks, VectorE
            # the rest plus the pos/cnt window passes.
            # reversed order: the VectorE-consumed tiles (i3, i2) go
            # first so ScalarE gets two chunk-times to drain its h0
            # backlog before its next PSUM tiles arrive
            for i in range(NI - 1, -1, -1):
                ps = pchunk_pool.tile([P, HC], f32, tag="chunk")
                dense(ps, i, xt23_t, 0, 0, start=True, stop=True)
                dense(ps, i, xt23_t, MMW, 1, start=True, stop=True)
                if i == 0:
                    consume_act(ps, 0, HC, 12)
                elif i == 1:
                    consume_act(ps, 0, MMW, 13)
                    consume_dve(ps, MMW, HC, 14)
                else:
                    consume_dve(ps, 0, HC, 13 + i)

            # ---- jc1 (slabs 4-7), all diff-class ----
            for i in range(NI):
                for h in range(2):
                    xt = xt45_t if h == 0 else xt67_t
                    ps = pchunk_pool.tile([P, HC], f32, tag="chunk")
                    dense(ps, i, xt, 0, 0, start=True, stop=True)
                    dense(ps, i, xt, MMW, 1, start=True, stop=True)
                    if h == 0:
                        consume_act(ps, 0, HC, 17 + i)
                    elif i < NI - 1:
                        consume_dve(ps, 0, HC, 21 + i)
                    else:
                        # last tile: split so the tail is one FD512 op
                        consume_act(ps, 0, MMW, 25)
                        consume_dve(ps, MMW, HC, 21 + i)
                        # most accumulator columns are final - overlap
                        # their DMA with the last consumers
                        nc.sync.dma_start(out=acc_ap[:, 0:21], in_=acc[:, 0:21])

            nc.sync.dma_start(out=acc_ap[:, 21:28], in_=acc[:, 21:28])

    nc.compile()
    return nc


def _get_compiled():
    global _compiled
    if _compiled is None:
        _compiled = _build()
    return _compiled


def _prep(inputs):
    import ml_dtypes

    x = np.asarray(inputs["inputs"], dtype=np.float32)
    t = np.asarray(inputs["targets"]).astype(np.int64)
    assert x.shape == (N, D)

    perm = np.argsort(t, kind="stable")
    xs, ts = x[perm], t[perm]
    counts = np.bincount(ts, minlength=NCLS)
    cstart = np.concatenate([[0], np.cumsum(counts)])

    xq = xs.astype(ml_dtypes.float8_e4m3)
    # K-plane-major PE view: kv[p, s, row] = xq[row, s*128 + p]
    kv = np.ascontiguousarray(xq.T.reshape(KS, P, N).transpose(1, 0, 2))

    in_maps = []
    meta = []
    cls_ar = np.arange(NCLS)
    for c in range(NCORES):
        rows = slice(c * R, (c + 1) * R)
        tloc = ts[rows]
        rot = c * R - LOFF          # rotation offset (may be negative)
        s_c = int(cstart[tloc[0]])
        e_c = int(cstart[tloc[-1] + 1])
        assert 0 <= s_c - rot and e_c - rot <= HC, \
            f"mask band outside [0,{HC}) on core {c}"
        for i in range(NI):
            lo = int(cstart[tloc[i * P]]) - rot
            hi = int(cstart[tloc[i * P + P - 1] + 1]) - rot
            assert WS[i] <= lo and hi <= WS[i] + W, \
                f"window overflow on core {c} i-tile {i}: [{lo},{hi})"
        cols = (rot + np.arange(N)) % N
        xrr_full = kv[:, :, cols]         # [128, 4, 4096]
        xr0a = np.ascontiguousarray(xrr_full[:, 0:2, 0:MMW])
        xr0b = np.ascontiguousarray(xrr_full[:, 2:4, 0:MMW])
        xr1 = np.ascontiguousarray(xrr_full[:, :, MMW:2 * MMW])
        xr23 = np.ascontiguousarray(xrr_full[:, :, 2 * MMW:4 * MMW])
        xr45 = np.ascontiguousarray(xrr_full[:, :, 4 * MMW:6 * MMW])
        xr67 = np.ascontiguousarray(xrr_full[:, :, 6 * MMW:8 * MMW])
        xl = np.ascontiguousarray(
            kv[:, :, c * R:(c + 1) * R].reshape(P, KS, NI, P)
            .transpose(2, 0, 1, 3))
        am = np.zeros((NCLS, R), dtype=ml_dtypes.float8_e5m2)
        am[tloc, np.arange(R)] = -SHIFT
        bcls = ts[cols[:HC]]
        b01 = (cls_ar[:, None] == bcls[None, :]).astype(ml_dtypes.float8_e5m2)
        in_maps.append({"xr0a": xr0a, "xr0b": xr0b, "xr1": xr1,
                        "xr23": xr23, "xr45": xr45, "xr67": xr67,
                        "xl": xl, "am": am,
                        "b01a": np.ascontiguousarray(b01[:, 0:MMW]),
                        "b01b": np.ascontiguousarray(b01[:, MMW:HC])})
        # neg counts per local row, in acc's [partition, i-tile] layout
        ncnt = (N - counts[tloc]).astype(np.float64).reshape(NI, P).T
        meta.append(ncnt)
    return in_maps, meta


def _reduce_results(res, meta):
    total = np.float64(0.0)
    for c in range(NCORES):
        a = np.asarray(res.results[c]["acc"], dtype=np.float64)  # [128, 28]
        pos_sum = -2.0 * (a[:, 0:4] + (SHIFT - 0.5) * W)
        pos_cnt = a[:, 4:8]
        # jc0-h1 pieces per i-tile: i0=col12(ACT), i1=col13(ACT)+col14
        # (DVE FD512), i2=col15, i3=col16 (DVE FD1024); DVE max-ops
        # carry a +FD/2 offset each
        neg24 = a[:, 24] + a[:, 25] - 0.5 * MMW
        neg_relu = a[:, 8:12] + a[:, 17:21] \
            + np.stack([a[:, 21] - 0.5 * HC, a[:, 22] - 0.5 * HC,
                        a[:, 23] - 0.5 * HC, neg24], axis=1) \
            + np.stack([
                a[:, 12],
                a[:, 13] + a[:, 14] - 0.5 * MMW,
                a[:, 15] - 0.5 * HC,
                a[:, 16] - 0.5 * HC,
            ], axis=1)
        pos_mean = pos_sum / np.maximum(pos_cnt, 1.0)
        neg_mean = 25.0 * neg_relu / meta[c]
        total += float(np.sum(pos_mean + neg_mean))
    return np.float32(total / N)


def kernel(**inputs) -> np.ndarray:
    from concourse.bass_utils import run_bass_kernel_spmd

    nc = _get_compiled()
    in_maps, meta = _prep(inputs)
    res = run_bass_kernel_spmd(nc, in_maps, list(range(NCORES)))
    return _reduce_results(res, meta)


def kernel_timed(**inputs):
    """Like kernel(), but NTFF-profiles core 0 and returns
    (loss, exec_time_ns, profile_json_path)."""
    from concourse.bass_utils import run_bass_kernel_spmd

    nc = _get_compiled()
    in_maps, meta = _prep(inputs)
    run_bass_kernel_spmd(nc, in_maps, list(range(NCORES)))  # warm NEFF cache
    res = run_bass_kernel_spmd(nc, in_maps, list(range(NCORES)), trace=True)
    return _reduce_results(res, meta), res.exec_time_ns, res.profile_json
